# revision 13
# baseline (speedup 1.0000x reference)
"""Trainium2 Bass kernel for nn_CrossModalFusion.

Math: with seq_len=1 on both attention sides, softmax over the single key is
identically 1, so MHA collapses to  ctx = x_kv @ Wv.T @ Wo.T + (Wo @ bv + bo).
We fuse (Wv.T @ Wo.T) into one [d, d] weight on the host, so each modality is a
single [B,d]x[d,d] matmul, a residual add, a LayerNorm, plus the final
concat([img_out, txt_out, img_out*txt_out]).

Sharding: pure data parallel over the batch dim across 8 NeuronCores, weights
replicated, no collectives.

Device data is fp16 (full PE rate like bf16, ~2^-11 rounding); the output is
stored as fp16 and cast to f32 on the host (saves 24 MiB/core of store
traffic). All input tensors are host-packed into SBUF-ready [128, big] slabs
(slab row p holds exactly partition p's bytes) so every DMA load is a plain 2D
slice with large contiguous descriptors.

Engine assignment (measured on this hw: gpsimd InstTensorScalarPtr is ~14us
per [128,1024] tile -- 10x the cost model -- so the LN normalize must NOT run
there):
  PE   : the two fused matmuls (fp16, 16 calls per 128-row b-tile)
  DVE  : residual add, bn_stats/bn_aggr, reciprocal, tiny -mu*rstd ops
  Act  : sqrt(var+eps), LN normalize via Identity(y*rstd + (-mu*rstd)) with
         per-partition scale/bias APs, writing fp16 straight into the fused
         output tile
  Pool : only the elementwise img_out*txt_out product (plain tensor_mul is
         fast on gpsimd; tensor_scalar is not)
  SP   : all DMA triggers by default (load/store rings selectable per variant)

Per-core layout (Bs = 4096 rows):
  - img_n/txt_n  packed naturals (residual input), loaded [128, 4096] per
                 512-row group
  - imgT/txtT    packed transposes (matmul stationary operand: the PE wants
                 the contraction dim on partitions), loaded [128, 4096]/chunk
  - w_it/w_ti    packed fused weights [d_in, d_out]
  - out          [Bs, 3d] f16, one fused [128, 3072] store per b-tile
"""

import numpy as np

B, D, NCORES = 32768, 1024, 8
BS = B // NCORES          # 4096 rows per core
PT = 128                  # partition tile (rows per b-tile)
NBT = BS // PT            # 32 b-tiles per core
KT = D // PT              # 8 k-tiles of the contraction
NH = 512                  # psum half width (one fp32 PSUM bank)
CHUNK = 512               # b-columns per transposed-input chunk load
NCHUNK = BS // CHUNK
NATCH = 512               # rows per natural-input chunk load (== CHUNK)
EPS = 1e-5
XDT = np.float16  # device dtype for x / weights (fp16: full PE rate)

_CACHE = {}


def _build_program(repeats, has_bias, has_affine, variant="full"):
    """variant: "<base>[:<load_eng>:<store_eng>]"
    base: full (== ps4: one PSUM bank per matmul half, DVE consumes half h
          while the PE streams half h+1) | ps2 (full-width PSUM tiles) |
          vecnomm (full minus matmuls) | mmonly | dmafull | dmaonly
    engines: sp | act | gp | dve
    """
    import concourse.bass as bass
    import concourse.tile as tile
    from concourse import bacc, mybir

    parts = (variant.split(":") + ["sp", "sp"])[:3]
    base, load_eng_name, store_eng_name = parts

    f32 = mybir.dt.float32
    f16 = mybir.dt.float16
    AF = mybir.ActivationFunctionType
    ALU = mybir.AluOpType

    nc = bacc.Bacc("TRN2", enable_partition_id=False)

    # packed layouts: loads are [128, big] row-contiguous slabs (row p of the
    # slab is exactly partition p's bytes)
    NATB = NATCH // PT
    img_n = nc.declare_dram_parameter(
        "img_n", [(BS // NATCH) * PT, NATB * D], f16, isOutput=False)
    txt_n = nc.declare_dram_parameter(
        "txt_n", [(BS // NATCH) * PT, NATB * D], f16, isOutput=False)
    imgT = nc.declare_dram_parameter(
        "imgT", [NCHUNK * PT, KT * CHUNK], f16, isOutput=False)
    txtT = nc.declare_dram_parameter(
        "txtT", [NCHUNK * PT, KT * CHUNK], f16, isOutput=False)
    w_it = nc.declare_dram_parameter("w_it", [PT, KT * D], f16, isOutput=False)
    w_ti = nc.declare_dram_parameter("w_ti", [PT, KT * D], f16, isOutput=False)
    bias_d = affine_d = None
    if has_bias:
        bias_d = nc.declare_dram_parameter("bias", [2, D], f32, isOutput=False)
    if has_affine:
        affine_d = nc.declare_dram_parameter("affine", [4, D], f32, isOutput=False)
    out_d = nc.declare_dram_parameter("out", [BS, 3 * D], f16, isOutput=True)

    do_mm = base in ("full", "ps4", "ps2", "mmonly", "dmafull")
    do_nat = base in ("full", "ps4", "ps2", "vecnomm", "dmafull")

    with tile.TileContext(nc) as tc:
        _engs = {"sp": nc.sync, "act": nc.scalar, "gp": nc.gpsimd,
                 "dve": nc.vector}
        load_e = _engs[load_eng_name]
        store_e = _engs[store_eng_name]
        with (
            tc.tile_pool(name="singles", bufs=1) as singles,
            tc.tile_pool(name="wpool", bufs=1) as wpool,
            tc.tile_pool(name="xtpool", bufs=3) as xtpool,
            tc.tile_pool(name="natpool", bufs=2) as natpool,
            tc.tile_pool(name="ypool", bufs=2) as ypool,
            tc.tile_pool(name="outpool", bufs=3) as outpool,
            tc.tile_pool(name="smalls", bufs=6) as smalls,
            tc.tile_pool(name="psum", bufs=2, space=bass.MemorySpace.PSUM) as psum,
        ):
            def body():
                eps_t = singles.tile([PT, 1], f32, tag="eps")
                nc.vector.memset(eps_t, EPS)
                negone = singles.tile([PT, 1], f32, tag="negone")
                nc.vector.memset(negone, -1.0)
                zps = None
                if base == "vecnomm":
                    zps = singles.tile([PT, D], f32, tag="zps")
                    nc.vector.memset(zps, 0.0)

                w_sb = {}
                if do_mm:
                    for mod, w_d in (("it", w_it), ("ti", w_ti)):
                        w = wpool.tile([PT, KT, D], f16, tag=f"w_{mod}",
                                       name=f"w_{mod}")
                        load_e.dma_start(
                            out=w, in_=w_d.rearrange("p (k n) -> p k n", k=KT))
                        w_sb[mod] = w

                bias_bc, aff_bc = {}, {}
                if has_bias:
                    for i, mod in enumerate(("it", "ti")):
                        t = singles.tile([PT, D], f32, tag=f"bias_{mod}",
                                         name=f"bias_{mod}")
                        s = bias_d[i : i + 1, :]
                        s = bass.AP(tensor=s.tensor, offset=s.offset,
                                    ap=[[0, PT], [1, D]])
                        load_e.dma_start(out=t, in_=s)
                        bias_bc[mod] = t
                if has_affine:
                    for i, nm in enumerate(("g_img", "b_img", "g_txt", "b_txt")):
                        t = singles.tile([PT, D], f32, tag=f"aff_{nm}", name=nm)
                        s = affine_d[i : i + 1, :]
                        s = bass.AP(tensor=s.tensor, offset=s.offset,
                                    ap=[[0, PT], [1, D]])
                        load_e.dma_start(out=t, in_=s)
                        aff_bc[nm] = t

                for c in range(NCHUNK):
                    xt_sb = {}
                    if do_mm:
                        for mod, xT_d in (("it", txtT), ("ti", imgT)):
                            # "it" makes img_ctx from txt; "ti" the reverse
                            xt = xtpool.tile([PT, KT, CHUNK], f16,
                                             tag=f"xt_{mod}", name=f"xt_{mod}")
                            load_e.dma_start(
                                out=xt,
                                in_=xT_d[c * PT : (c + 1) * PT, :].rearrange(
                                    "p (k b) -> p k b", k=KT))
                            xt_sb[mod] = xt
                    nat_sb = {}
                    if do_nat:
                        for mod, x_nat_d in (("it", img_n), ("ti", txt_n)):
                            # residual input: "it" normalizes img, "ti" txt
                            natg = natpool.tile([PT, NATB, D], f16,
                                                tag=f"nat_{mod}",
                                                name=f"nat_{mod}")
                            load_e.dma_start(
                                out=natg,
                                in_=x_nat_d[c * PT : (c + 1) * PT, :].rearrange(
                                    "p (a d) -> p a d", a=NATB))
                            nat_sb[mod] = natg

                    for bb in range(CHUNK // PT):
                        b0 = c * CHUNK + bb * PT
                        rows = slice(b0, b0 + PT)

                        if base == "mmonly":
                            for mod in ("it", "ti"):
                                ps = psum.tile([PT, D], f32, tag=f"ps_{mod}",
                                               name=f"ps_{mod}")
                                xt = xt_sb[mod]
                                for k in range(KT):
                                    lhsT = xt[:, k, bb * PT : (bb + 1) * PT]
                                    for h in range(2):
                                        nc.tensor.matmul(
                                            ps[:, h * NH : (h + 1) * NH],
                                            lhsT,
                                            w_sb[mod][:, k, h * NH : (h + 1) * NH],
                                            start=(k == 0),
                                            stop=(k == KT - 1))
                            continue

                        if base in ("dmaonly", "dmafull"):
                            dummy = outpool.tile([PT, 3, D], f16, tag="out",
                                                 name="out")
                            nc.vector.memset(dummy[:, 0:1, 0:1], 0.0)
                            store_e.dma_start(
                                out=out_d[rows, :].rearrange(
                                    "p (s d) -> p s d", s=3),
                                in_=dummy)
                            continue

                        outt = outpool.tile([PT, 3, D], f16, tag="out",
                                            name="out")
                        for mi, (mod, gb) in enumerate((
                                ("it", ("g_img", "b_img")),
                                ("ti", ("g_txt", "b_txt")))):
                            y = ypool.tile([PT, D], f32, tag=f"y_{mod}",
                                           name=f"y_{mod}")
                            stats = smalls.tile([PT, 2, 6], f32,
                                                tag=f"st_{mod}",
                                                name=f"st_{mod}")

                            if base == "vecnomm":
                                nc.vector.tensor_add(
                                    y, zps, nat_sb[mod][:, bb, :])
                                # (ps2 keeps the old full-width PSUM layout)
                                nc.vector.bn_stats(stats[:, 0, :], y[:, 0:NH])
                                nc.vector.bn_stats(stats[:, 1, :], y[:, NH:D])
                            elif base in ("full", "ps4"):
                                # one PSUM bank per half: DVE consumes half h
                                # while the PE still streams half h+1
                                xt = xt_sb[mod]
                                for h in range(2):
                                    ph = psum.tile([PT, NH], f32,
                                                   tag=f"ps_{mod}{h}",
                                                   name=f"ps_{mod}{h}")
                                    for k in range(KT):
                                        lhsT = xt[:, k, bb * PT : (bb + 1) * PT]
                                        nc.tensor.matmul(
                                            ph,
                                            lhsT,
                                            w_sb[mod][:, k, h * NH : (h + 1) * NH],
                                            start=(k == 0),
                                            stop=(k == KT - 1))
                                    nc.vector.tensor_add(
                                        y[:, h * NH : (h + 1) * NH], ph,
                                        nat_sb[mod][:, bb, h * NH : (h + 1) * NH])
                                    nc.vector.bn_stats(
                                        stats[:, h, :],
                                        y[:, h * NH : (h + 1) * NH])
                            else:
                                ps = psum.tile([PT, D], f32, tag=f"ps_{mod}",
                                               name=f"ps_{mod}")
                                xt = xt_sb[mod]
                                for k in range(KT):
                                    lhsT = xt[:, k, bb * PT : (bb + 1) * PT]
                                    for h in range(2):
                                        nc.tensor.matmul(
                                            ps[:, h * NH : (h + 1) * NH],
                                            lhsT,
                                            w_sb[mod][:, k, h * NH : (h + 1) * NH],
                                            start=(k == 0),
                                            stop=(k == KT - 1))

                                nc.vector.tensor_add(
                                    y, ps, nat_sb[mod][:, bb, :])
                                nc.vector.bn_stats(stats[:, 0, :], y[:, 0:NH])
                                nc.vector.bn_stats(stats[:, 1, :], y[:, NH:D])
                            if has_bias:
                                nc.gpsimd.tensor_add(y, y, bias_bc[mod])
                            mv = smalls.tile([PT, 2], f32, tag=f"mv_{mod}",
                                             name=f"mv_{mod}")
                            nc.vector.bn_aggr(mv, stats)

                            std = smalls.tile([PT, 1], f32, tag=f"sd_{mod}",
                                              name=f"sd_{mod}")
                            nc.scalar.activation(
                                std, mv[:, 1:2], func=AF.Sqrt, bias=eps_t,
                                scale=1.0)
                            rstd = smalls.tile([PT, 1], f32, tag=f"rs_{mod}",
                                               name=f"rs_{mod}")
                            nc.vector.reciprocal(rstd, std)
                            mur = smalls.tile([PT, 1], f32, tag=f"mu_{mod}",
                                              name=f"mu_{mod}")
                            nc.vector.tensor_mul(mur, mv[:, 0:1], rstd)
                            nmr = smalls.tile([PT, 1], f32, tag=f"nm_{mod}",
                                              name=f"nm_{mod}")
                            nc.vector.tensor_mul(nmr, mur, negone)

                            # LN normalize on Act: Identity(y*rstd - mu*rstd),
                            # fp16 straight into the fused output tile
                            sect = outt[:, mi, :]
                            nc.scalar.activation(
                                sect, y, func=AF.Identity, bias=nmr, scale=rstd)
                            if has_affine:
                                nc.gpsimd.tensor_mul(sect, sect, aff_bc[gb[0]])
                                nc.gpsimd.tensor_add(sect, sect, aff_bc[gb[1]])

                        nc.gpsimd.tensor_mul(
                            outt[:, 2, :], outt[:, 0, :], outt[:, 1, :])
                        store_e.dma_start(
                            out=out_d[rows, :].rearrange("p (s d) -> p s d", s=3),
                            in_=outt)

            if repeats == 1:
                body()
            elif repeats < 0:  # python-unrolled |repeats| iterations (sim only)
                for _ in range(-repeats):
                    body()
            else:
                with tc.For_i(0, repeats, 1):
                    body()

    nc.finalize()
    return nc


def _get_exec(repeats=1, has_bias=False, has_affine=False, variant="full"):
    key = (repeats, has_bias, has_affine, variant)
    if key in _CACHE:
        return _CACHE[key]

    import jax
    from jax.experimental.shard_map import shard_map
    from jax.sharding import Mesh, PartitionSpec
    from concourse import mybir
    from concourse.bass2jax import (
        _bass_exec_p,
        install_neuronx_cc_hook,
        partition_id_tensor,
    )

    install_neuronx_cc_hook()
    nc = _build_program(repeats, has_bias, has_affine, variant)

    partition_name = nc.partition_id_tensor.name if nc.partition_id_tensor else None
    in_names, out_names, out_avals = [], [], []
    for alloc in nc.m.functions[0].allocations:
        if not isinstance(alloc, mybir.MemoryLocationSet):
            continue
        name = alloc.memorylocations[0].name
        if alloc.kind == "ExternalInput":
            if name != partition_name:
                in_names.append(name)
        elif alloc.kind == "ExternalOutput":
            out_names.append(name)
            out_avals.append(
                jax.core.ShapedArray(tuple(alloc.tensor_shape), mybir.dt.np(alloc.dtype))
            )
    n_params = len(in_names)
    all_in_names = list(in_names) + out_names
    if partition_name is not None:
        all_in_names.append(partition_name)
    all_in_names = tuple(all_in_names)

    def _body(*args):
        operands = list(args)
        if partition_name is not None:
            operands.append(partition_id_tensor())
        return tuple(
            _bass_exec_p.bind(
                *operands,
                out_avals=tuple(out_avals),
                in_names=all_in_names,
                out_names=tuple(out_names),
                lowering_input_output_aliases=(),
                sim_require_finite=True,
                sim_require_nnan=True,
                nc=nc,
            )
        )

    devices = jax.devices()[:NCORES]
    assert len(devices) == NCORES, f"need {NCORES} devices, got {len(devices)}"
    mesh = Mesh(np.asarray(devices), ("core",))
    nspecs = n_params + len(out_names)
    fn = jax.jit(
        shard_map(
            _body,
            mesh=mesh,
            in_specs=(PartitionSpec("core"),) * nspecs,
            out_specs=(PartitionSpec("core"),) * len(out_names),
            check_rep=False,
        ),
        keep_unused=True,
    )
    entry = (fn, in_names, out_names, out_avals, mesh)
    _CACHE[key] = entry
    return entry


def _prep_inputs(inputs):
    """Host-side prep: fuse weights, cast, transpose. Returns (global input
    arrays dict keyed by dram param name, has_bias, has_affine)."""
    img = np.asarray(inputs["img"], np.float32)
    txt = np.asarray(inputs["txt"], np.float32)

    glob = {}
    has_bias = False
    bias_rows = []
    w_glob = {}
    for mod, wi, bi, wo, bo in (
        ("it", "Wi_it", "bi_it", "Wo_it", "bo_it"),
        ("ti", "Wi_ti", "bi_ti", "Wo_ti", "bo_ti"),
    ):
        Wi = np.asarray(inputs[wi], np.float32)
        Wo = np.asarray(inputs[wo], np.float32)
        bi = np.asarray(inputs[bi], np.float32)
        bo = np.asarray(inputs[bo], np.float32)
        Wv = Wi[2 * D : 3 * D]               # v = x_kv @ Wv.T + bv
        Wf = (Wv.T @ Wo.T).astype(XDT)      # ctx = x_kv @ Wf, [d_in, d_out]
        bf = Wo @ bi[2 * D : 3 * D] + bo
        w_glob[mod] = Wf
        bias_rows.append(bf)
        if np.any(bf != 0.0):
            has_bias = True

    aff = [np.asarray(inputs[k], np.float32)
           for k in ("g_img", "b_img", "g_txt", "b_txt")]
    has_affine = bool(
        np.any(aff[0] != 1.0) or np.any(aff[1] != 0.0)
        or np.any(aff[2] != 1.0) or np.any(aff[3] != 0.0)
    )

    img16 = img.astype(XDT)
    txt16 = txt.astype(XDT)
    NATB = NATCH // PT
    NG = BS // NATCH  # natural-chunk groups per core

    def pack_nat(x16):
        # [NCORES*BS, D] -> per-core [(NG*PT), NATB*D] slabs, concatenated:
        # row (g,p) holds rows g*NATCH + bb*PT + p for bb in range(NATB)
        r = x16.reshape(NCORES, NG, NATB, PT, D).transpose(0, 1, 3, 2, 4)
        return np.ascontiguousarray(r).reshape(NCORES * NG * PT, NATB * D)

    def pack_xT(x16):
        # per-core transposed [(NCHUNK*PT), KT*CHUNK]: row (c,p) holds
        # feature rows k*PT+p over batch-columns of chunk c
        xt = x16.reshape(NCORES, BS, D).transpose(0, 2, 1)  # [NC, D, BS]
        r = xt.reshape(NCORES, KT, PT, NCHUNK, CHUNK).transpose(0, 3, 2, 1, 4)
        return np.ascontiguousarray(r).reshape(NCORES * NCHUNK * PT, KT * CHUNK)

    def pack_w(w):
        r = w.reshape(KT, PT, D).transpose(1, 0, 2)
        return np.ascontiguousarray(r).reshape(PT, KT * D)

    glob["img_n"] = pack_nat(img16)
    glob["txt_n"] = pack_nat(txt16)
    glob["imgT"] = pack_xT(img16)
    glob["txtT"] = pack_xT(txt16)
    wpk_it = pack_w(w_glob["it"])
    wpk_ti = pack_w(w_glob["ti"])
    glob["w_it"] = np.broadcast_to(wpk_it, (NCORES, PT, KT * D)).reshape(NCORES * PT, KT * D).copy()
    glob["w_ti"] = np.broadcast_to(wpk_ti, (NCORES, PT, KT * D)).reshape(NCORES * PT, KT * D).copy()
    if has_bias:
        b = np.stack(bias_rows).astype(np.float32)  # [2, D]
        glob["bias"] = np.broadcast_to(b, (NCORES, 2, D)).reshape(NCORES * 2, D).copy()
    if has_affine:
        a = np.stack(aff).astype(np.float32)  # [4, D]
        glob["affine"] = np.broadcast_to(a, (NCORES, 4, D)).reshape(NCORES * 4, D).copy()
    return glob, has_bias, has_affine


def kernel(**inputs):
    glob, has_bias, has_affine = _prep_inputs(inputs)
    fn, in_names, out_names, out_avals, mesh = _get_exec(1, has_bias, has_affine)
    args = [glob[n] for n in in_names]
    zeros = [
        np.zeros((NCORES * av.shape[0], *av.shape[1:]), av.dtype) for av in out_avals
    ]
    outs = fn(*args, *zeros)
    return np.asarray(outs[0]).astype(np.float32)


if __name__ == "__main__":
    rng = np.random.default_rng(0)
    fake = {
        "img": rng.standard_normal((B, D), np.float32),
        "txt": rng.standard_normal((B, D), np.float32),
        "Wi_it": rng.standard_normal((3 * D, D), np.float32) / 32,
        "bi_it": np.zeros(3 * D, np.float32),
        "Wo_it": rng.standard_normal((D, D), np.float32) / 32,
        "bo_it": np.zeros(D, np.float32),
        "Wi_ti": rng.standard_normal((3 * D, D), np.float32) / 32,
        "bi_ti": np.zeros(3 * D, np.float32),
        "Wo_ti": rng.standard_normal((D, D), np.float32) / 32,
        "bo_ti": np.zeros(D, np.float32),
        "g_img": np.ones(D, np.float32),
        "b_img": np.zeros(D, np.float32),
        "g_txt": np.ones(D, np.float32),
        "b_txt": np.zeros(D, np.float32),
    }
    out = kernel(**fake)
    print(out.shape, out.dtype)


# revision 14
# speedup vs baseline: 1.6193x; 1.6193x over previous
"""Trainium2 Bass kernel for nn_CrossModalFusion.

Math: with seq_len=1 on both attention sides, softmax over the single key is
identically 1, so MHA collapses to  ctx = x_kv @ Wv.T @ Wo.T + (Wo @ bv + bo).
We fuse (Wv.T @ Wo.T) into one [d, d] weight on the host, so each modality is a
single [B,d]x[d,d] matmul, a residual add, a LayerNorm, plus the final
concat([img_out, txt_out, img_out*txt_out]).

Sharding: pure data parallel over the batch dim across 8 NeuronCores, weights
replicated, no collectives.

Device data is fp16 (full PE rate like bf16, ~2^-11 rounding); the output is
stored as fp16 and cast to f32 on the host (saves 24 MiB/core of store
traffic). All input tensors are host-packed into SBUF-ready [128, big] slabs
(slab row p holds exactly partition p's bytes) so every DMA load is a plain 2D
slice with large contiguous descriptors.

Engine assignment (measured on this hw: gpsimd InstTensorScalarPtr is ~14us
per [128,1024] tile -- 10x the cost model -- so the LN normalize must NOT run
there):
  PE   : the two fused matmuls (fp16, 16 calls per 128-row b-tile)
  DVE  : residual add, bn_stats/bn_aggr, reciprocal, tiny -mu*rstd ops
  Act  : sqrt(var+eps), LN normalize via Identity(y*rstd + (-mu*rstd)) with
         per-partition scale/bias APs, writing fp16 straight into the fused
         output tile
  Pool : only the elementwise img_out*txt_out product (plain tensor_mul is
         fast on gpsimd; tensor_scalar is not)
  SP   : all DMA triggers by default (load/store rings selectable per variant)

Per-core layout (Bs = 4096 rows):
  - img_n/txt_n  packed naturals (residual input), loaded [128, 4096] per
                 512-row group
  - imgT/txtT    packed transposes (matmul stationary operand: the PE wants
                 the contraction dim on partitions), loaded [128, 4096]/chunk
  - w_it/w_ti    packed fused weights [d_in, d_out]
  - out          [Bs, 3d] f16, one fused [128, 3072] store per b-tile
"""

import numpy as np

B, D, NCORES = 32768, 1024, 8
BS = B // NCORES          # 4096 rows per core
PT = 128                  # partition tile (rows per b-tile)
NBT = BS // PT            # 32 b-tiles per core
KT = D // PT              # 8 k-tiles of the contraction
NH = 512                  # psum half width (one fp32 PSUM bank)
CHUNK = 512               # b-columns per transposed-input chunk load
NCHUNK = BS // CHUNK
NATCH = 512               # rows per natural-input chunk load (== CHUNK)
EPS = 1e-5
XDT = np.float16  # device dtype for x / weights (fp16: full PE rate)

_CACHE = {}


def _build_program(repeats, has_bias, has_affine, variant="full"):
    """variant: "<base>[:<load_eng>:<store_eng>]"
    base: full (== ps4: one PSUM bank per matmul half, DVE consumes half h
          while the PE streams half h+1) | ps2 (full-width PSUM tiles) |
          vecnomm (full minus matmuls) | mmonly | dmafull | dmaonly
    engines: sp | act | gp | dve
    """
    import concourse.bass as bass
    import concourse.tile as tile
    from concourse import bacc, mybir

    parts = (variant.split(":") + ["sp", "sp"])[:3]
    base, load_eng_name, store_eng_name = parts

    f32 = mybir.dt.float32
    f16 = mybir.dt.float16
    AF = mybir.ActivationFunctionType

    nc = bacc.Bacc("TRN2", enable_partition_id=False)

    # packed layouts: loads are [128, big] row-contiguous slabs (row p of the
    # slab is exactly partition p's bytes)
    NATB = NATCH // PT
    img_n = nc.declare_dram_parameter(
        "img_n", [(BS // NATCH) * PT, NATB * D], f16, isOutput=False)
    txt_n = nc.declare_dram_parameter(
        "txt_n", [(BS // NATCH) * PT, NATB * D], f16, isOutput=False)
    imgT = nc.declare_dram_parameter(
        "imgT", [NCHUNK * PT, KT * CHUNK], f16, isOutput=False)
    txtT = nc.declare_dram_parameter(
        "txtT", [NCHUNK * PT, KT * CHUNK], f16, isOutput=False)
    w_it = nc.declare_dram_parameter("w_it", [PT, KT * D], f16, isOutput=False)
    w_ti = nc.declare_dram_parameter("w_ti", [PT, KT * D], f16, isOutput=False)
    bias_d = affine_d = None
    if has_bias:
        bias_d = nc.declare_dram_parameter("bias", [2, D], f32, isOutput=False)
    if has_affine:
        affine_d = nc.declare_dram_parameter("affine", [4, D], f32, isOutput=False)
    out_d = nc.declare_dram_parameter("out", [BS, 3 * D], f16, isOutput=True)

    do_mm = base in ("full", "ps4", "ps2", "mmonly", "dmafull")
    do_nat = base in ("full", "ps4", "ps2", "vecnomm", "dmafull")

    with tile.TileContext(nc) as tc:
        _engs = {"sp": nc.sync, "act": nc.scalar, "gp": nc.gpsimd,
                 "dve": nc.vector}
        load_e = _engs[load_eng_name]
        store_e = _engs[store_eng_name]
        with (
            tc.tile_pool(name="singles", bufs=1) as singles,
            tc.tile_pool(name="wpool", bufs=1) as wpool,
            tc.tile_pool(name="xtpool", bufs=3) as xtpool,
            tc.tile_pool(name="natpool", bufs=2) as natpool,
            tc.tile_pool(name="ypool", bufs=2) as ypool,
            tc.tile_pool(name="outpool", bufs=3) as outpool,
            tc.tile_pool(name="smalls", bufs=6) as smalls,
            tc.tile_pool(name="psum", bufs=2, space=bass.MemorySpace.PSUM) as psum,
        ):
            def body():
                eps_t = singles.tile([PT, 1], f32, tag="eps")
                nc.vector.memset(eps_t, EPS)
                negone = singles.tile([PT, 1], f32, tag="negone")
                nc.vector.memset(negone, -1.0)
                zps = None
                if base == "vecnomm":
                    zps = singles.tile([PT, D], f32, tag="zps")
                    nc.vector.memset(zps, 0.0)

                w_sb = {}
                if do_mm:
                    for mod, w_d in (("it", w_it), ("ti", w_ti)):
                        w = wpool.tile([PT, KT, D], f16, tag=f"w_{mod}",
                                       name=f"w_{mod}")
                        load_e.dma_start(
                            out=w, in_=w_d.rearrange("p (k n) -> p k n", k=KT))
                        w_sb[mod] = w

                bias_bc, aff_bc = {}, {}
                if has_bias:
                    for i, mod in enumerate(("it", "ti")):
                        t = singles.tile([PT, D], f32, tag=f"bias_{mod}",
                                         name=f"bias_{mod}")
                        s = bias_d[i : i + 1, :]
                        s = bass.AP(tensor=s.tensor, offset=s.offset,
                                    ap=[[0, PT], [1, D]])
                        load_e.dma_start(out=t, in_=s)
                        bias_bc[mod] = t
                if has_affine:
                    for i, nm in enumerate(("g_img", "b_img", "g_txt", "b_txt")):
                        t = singles.tile([PT, D], f32, tag=f"aff_{nm}", name=nm)
                        s = affine_d[i : i + 1, :]
                        s = bass.AP(tensor=s.tensor, offset=s.offset,
                                    ap=[[0, PT], [1, D]])
                        load_e.dma_start(out=t, in_=s)
                        aff_bc[nm] = t

                for c in range(NCHUNK):
                    xt_sb = {}
                    if do_mm:
                        for mod, xT_d in (("it", txtT), ("ti", imgT)):
                            # "it" makes img_ctx from txt; "ti" the reverse
                            xt = xtpool.tile([PT, KT, CHUNK], f16,
                                             tag=f"xt_{mod}", name=f"xt_{mod}")
                            load_e.dma_start(
                                out=xt,
                                in_=xT_d[c * PT : (c + 1) * PT, :].rearrange(
                                    "p (k b) -> p k b", k=KT))
                            xt_sb[mod] = xt
                    nat_sb = {}
                    if do_nat:
                        for mod, x_nat_d in (("it", img_n), ("ti", txt_n)):
                            # residual input: "it" normalizes img, "ti" txt
                            natg = natpool.tile([PT, NATB, D], f16,
                                                tag=f"nat_{mod}",
                                                name=f"nat_{mod}")
                            load_e.dma_start(
                                out=natg,
                                in_=x_nat_d[c * PT : (c + 1) * PT, :].rearrange(
                                    "p (a d) -> p a d", a=NATB))
                            nat_sb[mod] = natg

                    for bb in range(CHUNK // PT):
                        b0 = c * CHUNK + bb * PT
                        rows = slice(b0, b0 + PT)

                        if base == "mmonly":
                            for mod in ("it", "ti"):
                                ps = psum.tile([PT, D], f32, tag=f"ps_{mod}",
                                               name=f"ps_{mod}")
                                xt = xt_sb[mod]
                                for k in range(KT):
                                    lhsT = xt[:, k, bb * PT : (bb + 1) * PT]
                                    for h in range(2):
                                        nc.tensor.matmul(
                                            ps[:, h * NH : (h + 1) * NH],
                                            lhsT,
                                            w_sb[mod][:, k, h * NH : (h + 1) * NH],
                                            start=(k == 0),
                                            stop=(k == KT - 1))
                            continue

                        if base in ("dmaonly", "dmafull"):
                            dummy = outpool.tile([PT, 3, D], f16, tag="out",
                                                 name="out")
                            nc.vector.memset(dummy[:, 0:1, 0:1], 0.0)
                            store_e.dma_start(
                                out=out_d[rows, :].rearrange(
                                    "p (s d) -> p s d", s=3),
                                in_=dummy)
                            continue

                        outt = outpool.tile([PT, 3, D], f16, tag="out",
                                            name="out")
                        for mi, (mod, gb) in enumerate((
                                ("it", ("g_img", "b_img")),
                                ("ti", ("g_txt", "b_txt")))):
                            y = ypool.tile([PT, D], f32, tag=f"y_{mod}",
                                           name=f"y_{mod}")
                            stats = smalls.tile([PT, 2, 6], f32,
                                                tag=f"st_{mod}",
                                                name=f"st_{mod}")

                            if base == "vecnomm":
                                nc.vector.tensor_add(
                                    y, zps, nat_sb[mod][:, bb, :])
                                # (ps2 keeps the old full-width PSUM layout)
                                nc.vector.bn_stats(stats[:, 0, :], y[:, 0:NH])
                                nc.vector.bn_stats(stats[:, 1, :], y[:, NH:D])
                            elif base in ("full", "ps4"):
                                # one PSUM bank per half: DVE consumes half h
                                # while the PE still streams half h+1
                                xt = xt_sb[mod]
                                for h in range(2):
                                    ph = psum.tile([PT, NH], f32,
                                                   tag=f"ps_{mod}{h}",
                                                   name=f"ps_{mod}{h}")
                                    for k in range(KT):
                                        lhsT = xt[:, k, bb * PT : (bb + 1) * PT]
                                        nc.tensor.matmul(
                                            ph,
                                            lhsT,
                                            w_sb[mod][:, k, h * NH : (h + 1) * NH],
                                            start=(k == 0),
                                            stop=(k == KT - 1))
                                    nc.vector.tensor_add(
                                        y[:, h * NH : (h + 1) * NH], ph,
                                        nat_sb[mod][:, bb, h * NH : (h + 1) * NH])
                                    nc.vector.bn_stats(
                                        stats[:, h, :],
                                        y[:, h * NH : (h + 1) * NH])
                            else:
                                ps = psum.tile([PT, D], f32, tag=f"ps_{mod}",
                                               name=f"ps_{mod}")
                                xt = xt_sb[mod]
                                for k in range(KT):
                                    lhsT = xt[:, k, bb * PT : (bb + 1) * PT]
                                    for h in range(2):
                                        nc.tensor.matmul(
                                            ps[:, h * NH : (h + 1) * NH],
                                            lhsT,
                                            w_sb[mod][:, k, h * NH : (h + 1) * NH],
                                            start=(k == 0),
                                            stop=(k == KT - 1))

                                nc.vector.tensor_add(
                                    y, ps, nat_sb[mod][:, bb, :])
                                nc.vector.bn_stats(stats[:, 0, :], y[:, 0:NH])
                                nc.vector.bn_stats(stats[:, 1, :], y[:, NH:D])
                            if has_bias:
                                nc.gpsimd.tensor_add(y, y, bias_bc[mod])
                            mv = smalls.tile([PT, 2], f32, tag=f"mv_{mod}",
                                             name=f"mv_{mod}")
                            nc.vector.bn_aggr(mv, stats)

                            std = smalls.tile([PT, 1], f32, tag=f"sd_{mod}",
                                              name=f"sd_{mod}")
                            nc.scalar.activation(
                                std, mv[:, 1:2], func=AF.Sqrt, bias=eps_t,
                                scale=1.0)
                            rstd = smalls.tile([PT, 1], f32, tag=f"rs_{mod}",
                                               name=f"rs_{mod}")
                            nc.vector.reciprocal(rstd, std)
                            mur = smalls.tile([PT, 1], f32, tag=f"mu_{mod}",
                                              name=f"mu_{mod}")
                            nc.vector.tensor_mul(mur, mv[:, 0:1], rstd)
                            nmr = smalls.tile([PT, 1], f32, tag=f"nm_{mod}",
                                              name=f"nm_{mod}")
                            nc.vector.tensor_mul(nmr, mur, negone)

                            # LN normalize on Act: Identity(y*rstd - mu*rstd),
                            # fp16 straight into the fused output tile
                            sect = outt[:, mi, :]
                            nc.scalar.activation(
                                sect, y, func=AF.Identity, bias=nmr, scale=rstd)
                            if has_affine:
                                nc.gpsimd.tensor_mul(sect, sect, aff_bc[gb[0]])
                                nc.gpsimd.tensor_add(sect, sect, aff_bc[gb[1]])

                        nc.gpsimd.tensor_mul(
                            outt[:, 2, :], outt[:, 0, :], outt[:, 1, :])
                        store_e.dma_start(
                            out=out_d[rows, :].rearrange("p (s d) -> p s d", s=3),
                            in_=outt)

            if repeats == 1:
                body()
            elif repeats < 0:  # python-unrolled |repeats| iterations (sim only)
                for _ in range(-repeats):
                    body()
            else:
                with tc.For_i(0, repeats, 1):
                    body()

    nc.finalize()
    return nc


def _get_exec(repeats=1, has_bias=False, has_affine=False, variant="full"):
    key = (repeats, has_bias, has_affine, variant)
    if key in _CACHE:
        return _CACHE[key]

    import jax
    from jax.experimental.shard_map import shard_map
    from jax.sharding import Mesh, PartitionSpec
    from concourse import mybir
    from concourse.bass2jax import (
        _bass_exec_p,
        install_neuronx_cc_hook,
        partition_id_tensor,
    )

    install_neuronx_cc_hook()
    nc = _build_program(repeats, has_bias, has_affine, variant)

    partition_name = nc.partition_id_tensor.name if nc.partition_id_tensor else None
    in_names, out_names, out_avals = [], [], []
    for alloc in nc.m.functions[0].allocations:
        if not isinstance(alloc, mybir.MemoryLocationSet):
            continue
        name = alloc.memorylocations[0].name
        if alloc.kind == "ExternalInput":
            if name != partition_name:
                in_names.append(name)
        elif alloc.kind == "ExternalOutput":
            out_names.append(name)
            out_avals.append(
                jax.core.ShapedArray(tuple(alloc.tensor_shape), mybir.dt.np(alloc.dtype))
            )
    n_params = len(in_names)
    all_in_names = list(in_names) + out_names
    if partition_name is not None:
        all_in_names.append(partition_name)
    all_in_names = tuple(all_in_names)

    def _body(*args):
        operands = list(args)
        if partition_name is not None:
            operands.append(partition_id_tensor())
        return tuple(
            _bass_exec_p.bind(
                *operands,
                out_avals=tuple(out_avals),
                in_names=all_in_names,
                out_names=tuple(out_names),
                lowering_input_output_aliases=(),
                sim_require_finite=True,
                sim_require_nnan=True,
                nc=nc,
            )
        )

    devices = jax.devices()[:NCORES]
    assert len(devices) == NCORES, f"need {NCORES} devices, got {len(devices)}"
    mesh = Mesh(np.asarray(devices), ("core",))
    nspecs = n_params + len(out_names)
    fn = jax.jit(
        shard_map(
            _body,
            mesh=mesh,
            in_specs=(PartitionSpec("core"),) * nspecs,
            out_specs=(PartitionSpec("core"),) * len(out_names),
            check_rep=False,
        ),
        keep_unused=True,
    )
    entry = (fn, in_names, out_names, out_avals, mesh)
    _CACHE[key] = entry
    return entry


def _prep_inputs(inputs):
    """Host-side prep: fuse weights, cast, transpose. Returns (global input
    arrays dict keyed by dram param name, has_bias, has_affine)."""
    img = np.asarray(inputs["img"], np.float32)
    txt = np.asarray(inputs["txt"], np.float32)

    glob = {}
    has_bias = False
    bias_rows = []
    w_glob = {}
    for mod, wi, bi, wo, bo in (
        ("it", "Wi_it", "bi_it", "Wo_it", "bo_it"),
        ("ti", "Wi_ti", "bi_ti", "Wo_ti", "bo_ti"),
    ):
        Wi = np.asarray(inputs[wi], np.float32)
        Wo = np.asarray(inputs[wo], np.float32)
        bi = np.asarray(inputs[bi], np.float32)
        bo = np.asarray(inputs[bo], np.float32)
        Wv = Wi[2 * D : 3 * D]               # v = x_kv @ Wv.T + bv
        Wf = (Wv.T @ Wo.T).astype(XDT)      # ctx = x_kv @ Wf, [d_in, d_out]
        bf = Wo @ bi[2 * D : 3 * D] + bo
        w_glob[mod] = Wf
        bias_rows.append(bf)
        if np.any(bf != 0.0):
            has_bias = True

    aff = [np.asarray(inputs[k], np.float32)
           for k in ("g_img", "b_img", "g_txt", "b_txt")]
    has_affine = bool(
        np.any(aff[0] != 1.0) or np.any(aff[1] != 0.0)
        or np.any(aff[2] != 1.0) or np.any(aff[3] != 0.0)
    )

    img16 = img.astype(XDT)
    txt16 = txt.astype(XDT)
    NATB = NATCH // PT
    NG = BS // NATCH  # natural-chunk groups per core

    def pack_nat(x16):
        # [NCORES*BS, D] -> per-core [(NG*PT), NATB*D] slabs, concatenated:
        # row (g,p) holds rows g*NATCH + bb*PT + p for bb in range(NATB)
        r = x16.reshape(NCORES, NG, NATB, PT, D).transpose(0, 1, 3, 2, 4)
        return np.ascontiguousarray(r).reshape(NCORES * NG * PT, NATB * D)

    def pack_xT(x16):
        # per-core transposed [(NCHUNK*PT), KT*CHUNK]: row (c,p) holds
        # feature rows k*PT+p over batch-columns of chunk c
        xt = x16.reshape(NCORES, BS, D).transpose(0, 2, 1)  # [NC, D, BS]
        r = xt.reshape(NCORES, KT, PT, NCHUNK, CHUNK).transpose(0, 3, 2, 1, 4)
        return np.ascontiguousarray(r).reshape(NCORES * NCHUNK * PT, KT * CHUNK)

    def pack_w(w):
        r = w.reshape(KT, PT, D).transpose(1, 0, 2)
        return np.ascontiguousarray(r).reshape(PT, KT * D)

    glob["img_n"] = pack_nat(img16)
    glob["txt_n"] = pack_nat(txt16)
    glob["imgT"] = pack_xT(img16)
    glob["txtT"] = pack_xT(txt16)
    wpk_it = pack_w(w_glob["it"])
    wpk_ti = pack_w(w_glob["ti"])
    glob["w_it"] = np.broadcast_to(wpk_it, (NCORES, PT, KT * D)).reshape(NCORES * PT, KT * D).copy()
    glob["w_ti"] = np.broadcast_to(wpk_ti, (NCORES, PT, KT * D)).reshape(NCORES * PT, KT * D).copy()
    if has_bias:
        b = np.stack(bias_rows).astype(np.float32)  # [2, D]
        glob["bias"] = np.broadcast_to(b, (NCORES, 2, D)).reshape(NCORES * 2, D).copy()
    if has_affine:
        a = np.stack(aff).astype(np.float32)  # [4, D]
        glob["affine"] = np.broadcast_to(a, (NCORES, 4, D)).reshape(NCORES * 4, D).copy()
    return glob, has_bias, has_affine


def kernel(**inputs):
    glob, has_bias, has_affine = _prep_inputs(inputs)
    fn, in_names, out_names, out_avals, mesh = _get_exec(1, has_bias, has_affine)
    args = [glob[n] for n in in_names]
    zeros = [
        np.zeros((NCORES * av.shape[0], *av.shape[1:]), av.dtype) for av in out_avals
    ]
    outs = fn(*args, *zeros)
    return np.asarray(outs[0]).astype(np.float32)


if __name__ == "__main__":
    rng = np.random.default_rng(0)
    fake = {
        "img": rng.standard_normal((B, D), np.float32),
        "txt": rng.standard_normal((B, D), np.float32),
        "Wi_it": rng.standard_normal((3 * D, D), np.float32) / 32,
        "bi_it": np.zeros(3 * D, np.float32),
        "Wo_it": rng.standard_normal((D, D), np.float32) / 32,
        "bo_it": np.zeros(D, np.float32),
        "Wi_ti": rng.standard_normal((3 * D, D), np.float32) / 32,
        "bi_ti": np.zeros(3 * D, np.float32),
        "Wo_ti": rng.standard_normal((D, D), np.float32) / 32,
        "bo_ti": np.zeros(D, np.float32),
        "g_img": np.ones(D, np.float32),
        "b_img": np.zeros(D, np.float32),
        "g_txt": np.ones(D, np.float32),
        "b_txt": np.zeros(D, np.float32),
    }
    out = kernel(**fake)
    print(out.shape, out.dtype)


# revision 17
# speedup vs baseline: 1.9577x; 1.2090x over previous
"""Trainium2 Bass kernel for nn_CrossModalFusion.

Math: with seq_len=1 on both attention sides, softmax over the single key is
identically 1, so MHA collapses to  ctx = x_kv @ Wv.T @ Wo.T + (Wo @ bv + bo).
We fuse (Wv.T @ Wo.T) into one [d, d] weight on the host, so each modality is a
single [B,d]x[d,d] matmul, a residual add, a LayerNorm, plus the final
concat([img_out, txt_out, img_out*txt_out]).

Sharding: pure data parallel over the batch dim across 8 NeuronCores, weights
replicated, no collectives.

Device data is fp16 (full PE rate like bf16, ~2^-11 rounding); the output is
stored as fp16 and cast to f32 on the host (saves 24 MiB/core of store
traffic). All input tensors are host-packed into SBUF-ready [128, big] slabs
(slab row p holds exactly partition p's bytes) so every DMA load is a plain 2D
slice with large contiguous descriptors.

Engine assignment (measured on this hw: gpsimd InstTensorScalarPtr is ~14us
per [128,1024] tile -- 10x the cost model -- so the LN normalize must NOT run
there):
  PE   : the two fused matmuls (fp16, 16 calls per 128-row b-tile)
  DVE  : residual add, bn_stats/bn_aggr, reciprocal, tiny -mu*rstd ops
  Act  : sqrt(var+eps), LN normalize via Identity(y*rstd + (-mu*rstd)) with
         per-partition scale/bias APs, writing fp16 straight into the fused
         output tile
  Pool : only the elementwise img_out*txt_out product (plain tensor_mul is
         fast on gpsimd; tensor_scalar is not)
  SP   : all DMA triggers by default (load/store rings selectable per variant)

Per-core layout (Bs = 4096 rows):
  - img_n/txt_n  packed naturals (residual input), loaded [128, 4096] per
                 512-row group
  - imgT/txtT    packed transposes (matmul stationary operand: the PE wants
                 the contraction dim on partitions), loaded [128, 4096]/chunk
  - w_it/w_ti    packed fused weights [d_in, d_out]
  - out          [Bs, 3d] f16, one fused [128, 3072] store per b-tile
"""

import numpy as np

B, D, NCORES = 32768, 1024, 8
BS = B // NCORES          # 4096 rows per core
PT = 128                  # partition tile (rows per b-tile)
NBT = BS // PT            # 32 b-tiles per core
KT = D // PT              # 8 k-tiles of the contraction
NH = 512                  # psum half width (one fp32 PSUM bank)
CHUNK = 512               # b-columns per transposed-input chunk load
NCHUNK = BS // CHUNK
NATCH = 512               # rows per natural-input chunk load (== CHUNK)
EPS = 1e-5
XDT = np.float16  # device dtype for x / weights (fp16: full PE rate)

_CACHE = {}


def _build_program(repeats, has_bias, has_affine, variant="full"):
    """variant: "<base>[:<load_eng>:<store_eng>]"
    base: full (== ps4: one PSUM bank per matmul half, DVE consumes half h
          while the PE streams half h+1) | ps2 (full-width PSUM tiles) |
          vecnomm (full minus matmuls) | mmonly | dmafull | dmaonly
    engines: sp | act | gp | dve
    """
    import concourse.bass as bass
    import concourse.tile as tile
    from concourse import bacc, mybir

    parts = (variant.split(":") + ["sp", "sp"])[:3]
    base, load_eng_name, store_eng_name = parts

    f32 = mybir.dt.float32
    f16 = mybir.dt.float16
    AF = mybir.ActivationFunctionType

    nc = bacc.Bacc("TRN2", enable_partition_id=False)

    # packed layouts: loads are [128, big] row-contiguous slabs (row p of the
    # slab is exactly partition p's bytes)
    NATB = NATCH // PT
    img_n = nc.declare_dram_parameter(
        "img_n", [(BS // NATCH) * PT, NATB * D], f16, isOutput=False)
    txt_n = nc.declare_dram_parameter(
        "txt_n", [(BS // NATCH) * PT, NATB * D], f16, isOutput=False)
    imgT = nc.declare_dram_parameter(
        "imgT", [NCHUNK * PT, KT * CHUNK], f16, isOutput=False)
    txtT = nc.declare_dram_parameter(
        "txtT", [NCHUNK * PT, KT * CHUNK], f16, isOutput=False)
    w_it = nc.declare_dram_parameter("w_it", [PT, KT * D], f16, isOutput=False)
    w_ti = nc.declare_dram_parameter("w_ti", [PT, KT * D], f16, isOutput=False)
    bias_d = affine_d = None
    if has_bias:
        bias_d = nc.declare_dram_parameter("bias", [2, D], f32, isOutput=False)
    if has_affine:
        affine_d = nc.declare_dram_parameter("affine", [4, D], f32, isOutput=False)
    out_d = nc.declare_dram_parameter("out", [BS, 3 * D], f16, isOutput=True)

    do_mm = base in ("full", "ps4", "ps2", "mmonly", "dmafull")
    do_nat = base in ("full", "ps4", "ps2", "vecnomm", "dmafull")

    with tile.TileContext(nc) as tc:
        _engs = {"sp": nc.sync, "act": nc.scalar, "gp": nc.gpsimd,
                 "dve": nc.vector}
        load_e = _engs[load_eng_name]
        store_e = _engs[store_eng_name]
        with (
            tc.tile_pool(name="singles", bufs=1) as singles,
            tc.tile_pool(name="wpool", bufs=1) as wpool,
            tc.tile_pool(name="xtpool", bufs=3) as xtpool,
            tc.tile_pool(name="natpool", bufs=2) as natpool,
            tc.tile_pool(name="ypool", bufs=3) as ypool,
            tc.tile_pool(name="outpool", bufs=4) as outpool,
            tc.tile_pool(name="smalls", bufs=6) as smalls,
            tc.tile_pool(name="psum", bufs=2, space=bass.MemorySpace.PSUM) as psum,
        ):
            def body():
                eps_t = singles.tile([PT, 1], f32, tag="eps")
                nc.vector.memset(eps_t, EPS)
                negone = singles.tile([PT, 1], f32, tag="negone")
                nc.vector.memset(negone, -1.0)
                zps = None
                if base == "vecnomm":
                    zps = singles.tile([PT, D], f32, tag="zps")
                    nc.vector.memset(zps, 0.0)

                w_sb = {}
                if do_mm:
                    for mod, w_d in (("it", w_it), ("ti", w_ti)):
                        w = wpool.tile([PT, KT, D], f16, tag=f"w_{mod}",
                                       name=f"w_{mod}")
                        load_e.dma_start(
                            out=w, in_=w_d.rearrange("p (k n) -> p k n", k=KT))
                        w_sb[mod] = w

                bias_bc, aff_bc = {}, {}
                if has_bias:
                    for i, mod in enumerate(("it", "ti")):
                        t = singles.tile([PT, D], f32, tag=f"bias_{mod}",
                                         name=f"bias_{mod}")
                        s = bias_d[i : i + 1, :]
                        s = bass.AP(tensor=s.tensor, offset=s.offset,
                                    ap=[[0, PT], [1, D]])
                        load_e.dma_start(out=t, in_=s)
                        bias_bc[mod] = t
                if has_affine:
                    for i, nm in enumerate(("g_img", "b_img", "g_txt", "b_txt")):
                        t = singles.tile([PT, D], f32, tag=f"aff_{nm}", name=nm)
                        s = affine_d[i : i + 1, :]
                        s = bass.AP(tensor=s.tensor, offset=s.offset,
                                    ap=[[0, PT], [1, D]])
                        load_e.dma_start(out=t, in_=s)
                        aff_bc[nm] = t

                for c in range(NCHUNK):
                    xt_sb = {}
                    if do_mm:
                        for mod, xT_d in (("it", txtT), ("ti", imgT)):
                            # "it" makes img_ctx from txt; "ti" the reverse
                            xt = xtpool.tile([PT, KT, CHUNK], f16,
                                             tag=f"xt_{mod}", name=f"xt_{mod}")
                            load_e.dma_start(
                                out=xt,
                                in_=xT_d[c * PT : (c + 1) * PT, :].rearrange(
                                    "p (k b) -> p k b", k=KT))
                            xt_sb[mod] = xt
                    nat_sb = {}
                    if do_nat:
                        for mod, x_nat_d in (("it", img_n), ("ti", txt_n)):
                            # residual input: "it" normalizes img, "ti" txt
                            natg = natpool.tile([PT, NATB, D], f16,
                                                tag=f"nat_{mod}",
                                                name=f"nat_{mod}")
                            load_e.dma_start(
                                out=natg,
                                in_=x_nat_d[c * PT : (c + 1) * PT, :].rearrange(
                                    "p (a d) -> p a d", a=NATB))
                            nat_sb[mod] = natg

                    for bb in range(CHUNK // PT):
                        b0 = c * CHUNK + bb * PT
                        rows = slice(b0, b0 + PT)

                        if base == "mmonly":
                            for mod in ("it", "ti"):
                                ps = psum.tile([PT, D], f32, tag=f"ps_{mod}",
                                               name=f"ps_{mod}")
                                xt = xt_sb[mod]
                                for k in range(KT):
                                    lhsT = xt[:, k, bb * PT : (bb + 1) * PT]
                                    for h in range(2):
                                        nc.tensor.matmul(
                                            ps[:, h * NH : (h + 1) * NH],
                                            lhsT,
                                            w_sb[mod][:, k, h * NH : (h + 1) * NH],
                                            start=(k == 0),
                                            stop=(k == KT - 1))
                            continue

                        if base in ("dmaonly", "dmafull"):
                            dummy = outpool.tile([PT, 3, D], f16, tag="out",
                                                 name="out")
                            nc.vector.memset(dummy[:, 0:1, 0:1], 0.0)
                            store_e.dma_start(
                                out=out_d[rows, :].rearrange(
                                    "p (s d) -> p s d", s=3),
                                in_=dummy)
                            continue

                        outt = outpool.tile([PT, 3, D], f16, tag="out",
                                            name="out")
                        for mi, (mod, gb) in enumerate((
                                ("it", ("g_img", "b_img")),
                                ("ti", ("g_txt", "b_txt")))):
                            y = ypool.tile([PT, D], f32, tag=f"y_{mod}",
                                           name=f"y_{mod}")
                            stats = smalls.tile([PT, 2, 6], f32,
                                                tag=f"st_{mod}",
                                                name=f"st_{mod}")

                            if base == "vecnomm":
                                nc.vector.tensor_add(
                                    y, zps, nat_sb[mod][:, bb, :])
                                # (ps2 keeps the old full-width PSUM layout)
                                nc.vector.bn_stats(stats[:, 0, :], y[:, 0:NH])
                                nc.vector.bn_stats(stats[:, 1, :], y[:, NH:D])
                            elif base in ("full", "ps4"):
                                # one PSUM bank per half: DVE consumes half h
                                # while the PE still streams half h+1
                                xt = xt_sb[mod]
                                for h in range(2):
                                    ph = psum.tile([PT, NH], f32,
                                                   tag=f"ps_{mod}{h}",
                                                   name=f"ps_{mod}{h}")
                                    for k in range(KT):
                                        lhsT = xt[:, k, bb * PT : (bb + 1) * PT]
                                        nc.tensor.matmul(
                                            ph,
                                            lhsT,
                                            w_sb[mod][:, k, h * NH : (h + 1) * NH],
                                            start=(k == 0),
                                            stop=(k == KT - 1))
                                    nc.vector.tensor_add(
                                        y[:, h * NH : (h + 1) * NH], ph,
                                        nat_sb[mod][:, bb, h * NH : (h + 1) * NH])
                                    nc.vector.bn_stats(
                                        stats[:, h, :],
                                        y[:, h * NH : (h + 1) * NH])
                            else:
                                ps = psum.tile([PT, D], f32, tag=f"ps_{mod}",
                                               name=f"ps_{mod}")
                                xt = xt_sb[mod]
                                for k in range(KT):
                                    lhsT = xt[:, k, bb * PT : (bb + 1) * PT]
                                    for h in range(2):
                                        nc.tensor.matmul(
                                            ps[:, h * NH : (h + 1) * NH],
                                            lhsT,
                                            w_sb[mod][:, k, h * NH : (h + 1) * NH],
                                            start=(k == 0),
                                            stop=(k == KT - 1))

                                nc.vector.tensor_add(
                                    y, ps, nat_sb[mod][:, bb, :])
                                nc.vector.bn_stats(stats[:, 0, :], y[:, 0:NH])
                                nc.vector.bn_stats(stats[:, 1, :], y[:, NH:D])
                            if has_bias:
                                nc.gpsimd.tensor_add(y, y, bias_bc[mod])
                            mv = smalls.tile([PT, 2], f32, tag=f"mv_{mod}",
                                             name=f"mv_{mod}")
                            nc.vector.bn_aggr(mv, stats)

                            std = smalls.tile([PT, 1], f32, tag=f"sd_{mod}",
                                              name=f"sd_{mod}")
                            nc.scalar.activation(
                                std, mv[:, 1:2], func=AF.Sqrt, bias=eps_t,
                                scale=1.0)
                            rstd = smalls.tile([PT, 1], f32, tag=f"rs_{mod}",
                                               name=f"rs_{mod}")
                            nc.vector.reciprocal(rstd, std)
                            mur = smalls.tile([PT, 1], f32, tag=f"mu_{mod}",
                                              name=f"mu_{mod}")
                            nc.vector.tensor_mul(mur, mv[:, 0:1], rstd)
                            nmr = smalls.tile([PT, 1], f32, tag=f"nm_{mod}",
                                              name=f"nm_{mod}")
                            nc.vector.tensor_mul(nmr, mur, negone)

                            # LN normalize on Act: Identity(y*rstd - mu*rstd),
                            # fp16 straight into the fused output tile
                            sect = outt[:, mi, :]
                            nc.scalar.activation(
                                sect, y, func=AF.Identity, bias=nmr, scale=rstd)
                            if has_affine:
                                nc.gpsimd.tensor_mul(sect, sect, aff_bc[gb[0]])
                                nc.gpsimd.tensor_add(sect, sect, aff_bc[gb[1]])

                        nc.gpsimd.tensor_mul(
                            outt[:, 2, :], outt[:, 0, :], outt[:, 1, :])
                        store_e.dma_start(
                            out=out_d[rows, :].rearrange("p (s d) -> p s d", s=3),
                            in_=outt)

            if repeats == 1:
                body()
            elif repeats < 0:  # python-unrolled |repeats| iterations (sim only)
                for _ in range(-repeats):
                    body()
            else:
                with tc.For_i(0, repeats, 1):
                    body()

    nc.finalize()
    return nc


def _get_exec(repeats=1, has_bias=False, has_affine=False, variant="full"):
    key = (repeats, has_bias, has_affine, variant)
    if key in _CACHE:
        return _CACHE[key]

    import jax
    from jax.experimental.shard_map import shard_map
    from jax.sharding import Mesh, PartitionSpec
    from concourse import mybir
    from concourse.bass2jax import (
        _bass_exec_p,
        install_neuronx_cc_hook,
        partition_id_tensor,
    )

    install_neuronx_cc_hook()
    nc = _build_program(repeats, has_bias, has_affine, variant)

    partition_name = nc.partition_id_tensor.name if nc.partition_id_tensor else None
    in_names, out_names, out_avals = [], [], []
    for alloc in nc.m.functions[0].allocations:
        if not isinstance(alloc, mybir.MemoryLocationSet):
            continue
        name = alloc.memorylocations[0].name
        if alloc.kind == "ExternalInput":
            if name != partition_name:
                in_names.append(name)
        elif alloc.kind == "ExternalOutput":
            out_names.append(name)
            out_avals.append(
                jax.core.ShapedArray(tuple(alloc.tensor_shape), mybir.dt.np(alloc.dtype))
            )
    n_params = len(in_names)
    all_in_names = list(in_names) + out_names
    if partition_name is not None:
        all_in_names.append(partition_name)
    all_in_names = tuple(all_in_names)

    def _body(*args):
        operands = list(args)
        if partition_name is not None:
            operands.append(partition_id_tensor())
        return tuple(
            _bass_exec_p.bind(
                *operands,
                out_avals=tuple(out_avals),
                in_names=all_in_names,
                out_names=tuple(out_names),
                lowering_input_output_aliases=(),
                sim_require_finite=True,
                sim_require_nnan=True,
                nc=nc,
            )
        )

    devices = jax.devices()[:NCORES]
    assert len(devices) == NCORES, f"need {NCORES} devices, got {len(devices)}"
    mesh = Mesh(np.asarray(devices), ("core",))
    nspecs = n_params + len(out_names)
    fn = jax.jit(
        shard_map(
            _body,
            mesh=mesh,
            in_specs=(PartitionSpec("core"),) * nspecs,
            out_specs=(PartitionSpec("core"),) * len(out_names),
            check_rep=False,
        ),
        keep_unused=True,
    )
    entry = (fn, in_names, out_names, out_avals, mesh)
    _CACHE[key] = entry
    return entry


def _prep_inputs(inputs):
    """Host-side prep: fuse weights, cast, transpose. Returns (global input
    arrays dict keyed by dram param name, has_bias, has_affine)."""
    img = np.asarray(inputs["img"], np.float32)
    txt = np.asarray(inputs["txt"], np.float32)

    glob = {}
    has_bias = False
    bias_rows = []
    w_glob = {}
    for mod, wi, bi, wo, bo in (
        ("it", "Wi_it", "bi_it", "Wo_it", "bo_it"),
        ("ti", "Wi_ti", "bi_ti", "Wo_ti", "bo_ti"),
    ):
        Wi = np.asarray(inputs[wi], np.float32)
        Wo = np.asarray(inputs[wo], np.float32)
        bi = np.asarray(inputs[bi], np.float32)
        bo = np.asarray(inputs[bo], np.float32)
        Wv = Wi[2 * D : 3 * D]               # v = x_kv @ Wv.T + bv
        Wf = (Wv.T @ Wo.T).astype(XDT)      # ctx = x_kv @ Wf, [d_in, d_out]
        bf = Wo @ bi[2 * D : 3 * D] + bo
        w_glob[mod] = Wf
        bias_rows.append(bf)
        if np.any(bf != 0.0):
            has_bias = True

    aff = [np.asarray(inputs[k], np.float32)
           for k in ("g_img", "b_img", "g_txt", "b_txt")]
    has_affine = bool(
        np.any(aff[0] != 1.0) or np.any(aff[1] != 0.0)
        or np.any(aff[2] != 1.0) or np.any(aff[3] != 0.0)
    )

    img16 = img.astype(XDT)
    txt16 = txt.astype(XDT)
    NATB = NATCH // PT
    NG = BS // NATCH  # natural-chunk groups per core

    def pack_nat(x16):
        # [NCORES*BS, D] -> per-core [(NG*PT), NATB*D] slabs, concatenated:
        # row (g,p) holds rows g*NATCH + bb*PT + p for bb in range(NATB)
        r = x16.reshape(NCORES, NG, NATB, PT, D).transpose(0, 1, 3, 2, 4)
        return np.ascontiguousarray(r).reshape(NCORES * NG * PT, NATB * D)

    def pack_xT(x16):
        # per-core transposed [(NCHUNK*PT), KT*CHUNK]: row (c,p) holds
        # feature rows k*PT+p over batch-columns of chunk c
        xt = x16.reshape(NCORES, BS, D).transpose(0, 2, 1)  # [NC, D, BS]
        r = xt.reshape(NCORES, KT, PT, NCHUNK, CHUNK).transpose(0, 3, 2, 1, 4)
        return np.ascontiguousarray(r).reshape(NCORES * NCHUNK * PT, KT * CHUNK)

    def pack_w(w):
        r = w.reshape(KT, PT, D).transpose(1, 0, 2)
        return np.ascontiguousarray(r).reshape(PT, KT * D)

    glob["img_n"] = pack_nat(img16)
    glob["txt_n"] = pack_nat(txt16)
    glob["imgT"] = pack_xT(img16)
    glob["txtT"] = pack_xT(txt16)
    wpk_it = pack_w(w_glob["it"])
    wpk_ti = pack_w(w_glob["ti"])
    glob["w_it"] = np.broadcast_to(wpk_it, (NCORES, PT, KT * D)).reshape(NCORES * PT, KT * D).copy()
    glob["w_ti"] = np.broadcast_to(wpk_ti, (NCORES, PT, KT * D)).reshape(NCORES * PT, KT * D).copy()
    if has_bias:
        b = np.stack(bias_rows).astype(np.float32)  # [2, D]
        glob["bias"] = np.broadcast_to(b, (NCORES, 2, D)).reshape(NCORES * 2, D).copy()
    if has_affine:
        a = np.stack(aff).astype(np.float32)  # [4, D]
        glob["affine"] = np.broadcast_to(a, (NCORES, 4, D)).reshape(NCORES * 4, D).copy()
    return glob, has_bias, has_affine


def kernel(**inputs):
    glob, has_bias, has_affine = _prep_inputs(inputs)
    fn, in_names, out_names, out_avals, mesh = _get_exec(1, has_bias, has_affine)
    args = [glob[n] for n in in_names]
    zeros = [
        np.zeros((NCORES * av.shape[0], *av.shape[1:]), av.dtype) for av in out_avals
    ]
    outs = fn(*args, *zeros)
    return np.asarray(outs[0]).astype(np.float32)


if __name__ == "__main__":
    rng = np.random.default_rng(0)
    fake = {
        "img": rng.standard_normal((B, D), np.float32),
        "txt": rng.standard_normal((B, D), np.float32),
        "Wi_it": rng.standard_normal((3 * D, D), np.float32) / 32,
        "bi_it": np.zeros(3 * D, np.float32),
        "Wo_it": rng.standard_normal((D, D), np.float32) / 32,
        "bo_it": np.zeros(D, np.float32),
        "Wi_ti": rng.standard_normal((3 * D, D), np.float32) / 32,
        "bi_ti": np.zeros(3 * D, np.float32),
        "Wo_ti": rng.standard_normal((D, D), np.float32) / 32,
        "bo_ti": np.zeros(D, np.float32),
        "g_img": np.ones(D, np.float32),
        "b_img": np.zeros(D, np.float32),
        "g_txt": np.ones(D, np.float32),
        "b_txt": np.zeros(D, np.float32),
    }
    out = kernel(**fake)
    print(out.shape, out.dtype)
